# revision 3
# baseline (speedup 1.0000x reference)
"""NonLocalBlock1D (B=8, C=512, CI=256, L=2048) on 8 trn2 NeuronCores.

Data-parallel over batch: core b computes batch element b entirely on-chip.

Per-core math (x: [C, L]):
    theta = theta_w @ x + theta_b        [CI, L]
    phi   = phi_w @ x + phi_b            [CI, L]
    gT    = x^T @ g_w^T                  [L, CI]   (g bias folded into b2)
    fT[k, q]  = sum_d phi[d, k] theta[d, q]        (logits, transposed)
    e     = exp(fT)                       (no max subtraction; logits are O(10))
    s[q]  = sum_k e[k, q]                 (via all-ones stationary matmul)
    yT[d, q] = sum_k gT[k, d] e[k, q]
    out[c, l] = (out_w^T[d, c] . yT[d, l]) / s[l] + b2[c] + x[c, l]
where b2 = out_w @ g_b + out_b  (host-precomputed).

All matmuls run in float32r (full PE rate, ~1e-4 rel err). Weights are
pre-transposed on the host so no on-device transposes are needed.
"""

import numpy as np

import concourse.bass as bass
import concourse.tile as tile
from concourse import bacc, mybir
from concourse.bass_utils import run_bass_kernel_spmd

B, C, CI, L = 8, 512, 256, 2048
P = 128
CT = C // P      # 4 c-tiles
DT = CI // P     # 2 d-tiles
KT = L // P      # 16 k-tiles
QW = 512         # q-chunk width
QC = L // QW     # 4 q-chunks

F32 = mybir.dt.float32
F32R = mybir.dt.float32r
Exp = mybir.ActivationFunctionType.Exp
Copy = mybir.ActivationFunctionType.Copy
Ident = mybir.ActivationFunctionType.Identity

_CACHE = {}


def _build():
    nc = bacc.Bacc("TRN2", target_bir_lowering=False, debug=False)

    x_d = nc.dram_tensor("x", [C, L], F32R, kind="ExternalInput")
    thetaT_d = nc.dram_tensor("thetaT", [C, CI], F32R, kind="ExternalInput")
    phiT_d = nc.dram_tensor("phiT", [C, CI], F32R, kind="ExternalInput")
    gT_d = nc.dram_tensor("gT", [C, CI], F32R, kind="ExternalInput")
    outT_d = nc.dram_tensor("outT", [CI, C], F32R, kind="ExternalInput")
    theta_b_d = nc.dram_tensor("theta_b", [CI, 1], F32, kind="ExternalInput")
    phi_b_d = nc.dram_tensor("phi_b", [CI, 1], F32, kind="ExternalInput")
    b2_d = nc.dram_tensor("b2", [C, 1], F32, kind="ExternalInput")
    ones_d = nc.dram_tensor("ones", [P, P], F32R, kind="ExternalInput")
    out_d = nc.dram_tensor("out", [C, L], F32, kind="ExternalOutput")

    with tile.TileContext(nc) as tc:
        with tc.tile_pool(name="big", bufs=1) as big, \
             tc.tile_pool(name="wpool", bufs=1) as wpool, \
             tc.tile_pool(name="expp", bufs=4) as expp, \
             tc.tile_pool(name="ytp", bufs=2) as ytp, \
             tc.tile_pool(name="outp", bufs=4) as outp, \
             tc.tile_pool(name="smallp", bufs=2) as smallp, \
             tc.tile_pool(name="ps_proj", bufs=2, space="PSUM") as ps_proj, \
             tc.tile_pool(name="ps_ft", bufs=2, space="PSUM") as ps_ft, \
             tc.tile_pool(name="ps_sy", bufs=1, space="PSUM") as ps_sy, \
             tc.tile_pool(name="ps_out", bufs=1, space="PSUM") as ps_out:

            # ---- load x and weights into SBUF ----
            x_sb = []
            for ct in range(CT):
                xt = big.tile([P, L], F32R, name=f"x_sb{ct}", tag=f"x{ct}")
                nc.sync.dma_start(out=xt, in_=x_d.ap()[ct * P:(ct + 1) * P, :])
                x_sb.append(xt)

            thetaT_sb, phiT_sb, gTw_sb = [], [], []
            for ct in range(CT):
                tw = wpool.tile([P, CI], F32R, name=f"thw{ct}", tag=f"thw{ct}")
                nc.sync.dma_start(out=tw, in_=thetaT_d.ap()[ct * P:(ct + 1) * P, :])
                thetaT_sb.append(tw)
                pw = wpool.tile([P, CI], F32R, name=f"phw{ct}", tag=f"phw{ct}")
                nc.sync.dma_start(out=pw, in_=phiT_d.ap()[ct * P:(ct + 1) * P, :])
                phiT_sb.append(pw)
                gw = wpool.tile([P, CI], F32R, name=f"gw{ct}", tag=f"gw{ct}")
                nc.sync.dma_start(out=gw, in_=gT_d.ap()[ct * P:(ct + 1) * P, :])
                gTw_sb.append(gw)

            outT_sb = []
            for dt in range(DT):
                ow = wpool.tile([P, C], F32R, name=f"ow{dt}", tag=f"ow{dt}")
                nc.sync.dma_start(out=ow, in_=outT_d.ap()[dt * P:(dt + 1) * P, :])
                outT_sb.append(ow)

            theta_b_sb, phi_b_sb = [], []
            for dt in range(DT):
                tb = wpool.tile([P, 1], F32, name=f"tb{dt}", tag=f"tb{dt}")
                nc.sync.dma_start(out=tb, in_=theta_b_d.ap()[dt * P:(dt + 1) * P, :])
                theta_b_sb.append(tb)
                pb = wpool.tile([P, 1], F32, name=f"pb{dt}", tag=f"pb{dt}")
                nc.sync.dma_start(out=pb, in_=phi_b_d.ap()[dt * P:(dt + 1) * P, :])
                phi_b_sb.append(pb)
            b2_sb = []
            for ct in range(CT):
                bt = wpool.tile([P, 1], F32, name=f"b2{ct}", tag=f"b2{ct}")
                nc.sync.dma_start(out=bt, in_=b2_d.ap()[ct * P:(ct + 1) * P, :])
                b2_sb.append(bt)
            ones_sb = wpool.tile([P, P], F32R, name="ones_sb", tag="ones")
            nc.sync.dma_start(out=ones_sb, in_=ones_d.ap())

            # ---- projections ----
            # theta/phi: [CI, L], psum tile per (dt, chunk), contract over ct
            theta_sb = [big.tile([P, L], F32R, name=f"th_sb{dt}", tag=f"th{dt}")
                        for dt in range(DT)]
            phi_sb = [big.tile([P, L], F32R, name=f"ph_sb{dt}", tag=f"ph{dt}")
                      for dt in range(DT)]
            for dt in range(DT):
                for qc in range(QC):
                    qs = slice(qc * QW, (qc + 1) * QW)
                    pth = ps_proj.tile([P, QW], F32, name="pth", tag="ps_proj")
                    for ct in range(CT):
                        nc.tensor.matmul(
                            pth, thetaT_sb[ct][:, dt * P:(dt + 1) * P],
                            x_sb[ct][:, qs], start=(ct == 0), stop=(ct == CT - 1))
                    nc.scalar.activation(out=theta_sb[dt][:, qs], in_=pth,
                                         func=Ident, bias=theta_b_sb[dt], scale=1.0)
                    pph = ps_proj.tile([P, QW], F32, name="pph", tag="ps_proj")
                    for ct in range(CT):
                        nc.tensor.matmul(
                            pph, phiT_sb[ct][:, dt * P:(dt + 1) * P],
                            x_sb[ct][:, qs], start=(ct == 0), stop=(ct == CT - 1))
                    nc.scalar.activation(out=phi_sb[dt][:, qs], in_=pph,
                                         func=Ident, bias=phi_b_sb[dt], scale=1.0)

            # g_xT: [L, CI] per l-tile, x as stationary
            gT_sb = [big.tile([P, CI], F32R, name=f"gt_sb{lt}", tag=f"gt{lt}")
                     for lt in range(KT)]
            for lt in range(KT):
                pg = ps_proj.tile([P, CI], F32, name="pg", tag="ps_proj")
                for ct in range(CT):
                    nc.tensor.matmul(
                        pg, x_sb[ct][:, lt * P:(lt + 1) * P], gTw_sb[ct],
                        start=(ct == 0), stop=(ct == CT - 1))
                nc.vector.tensor_copy(gT_sb[lt], pg)

            # ---- attention, per q-chunk ----
            for qc in range(QC):
                qs = slice(qc * QW, (qc + 1) * QW)
                s_ps = ps_sy.tile([P, QW], F32, name="s_ps", tag="s")
                y_ps = [ps_sy.tile([P, QW], F32, name=f"y_ps{dt}", tag=f"y{dt}")
                        for dt in range(DT)]
                for kt in range(KT):
                    ks = slice(kt * P, (kt + 1) * P)
                    ft = ps_ft.tile([P, QW], F32, name="ft", tag="ft")
                    for dt in range(DT):
                        nc.tensor.matmul(ft, phi_sb[dt][:, ks],
                                         theta_sb[dt][:, qs],
                                         start=(dt == 0), stop=(dt == DT - 1))
                    ef = expp.tile([P, QW], F32R, name="ef", tag="ef")
                    nc.scalar.activation(out=ef, in_=ft, func=Exp)
                    nc.tensor.matmul(s_ps, ones_sb, ef,
                                     start=(kt == 0), stop=(kt == KT - 1))
                    for dt in range(DT):
                        nc.tensor.matmul(y_ps[dt],
                                         gT_sb[kt][:, dt * P:(dt + 1) * P], ef,
                                         start=(kt == 0), stop=(kt == KT - 1))

                recip = smallp.tile([P, QW], F32, name="recip", tag="recip")
                nc.vector.reciprocal(recip, s_ps)
                yT_sb = [ytp.tile([P, QW], F32R, name=f"yt{dt}", tag=f"yt{dt}")
                         for dt in range(DT)]
                for dt in range(DT):
                    nc.scalar.activation(out=yT_sb[dt], in_=y_ps[dt], func=Copy)

                # out projection for this q-chunk
                for ct in range(CT):
                    po = ps_out.tile([P, QW], F32, name="po", tag="po")
                    for dt in range(DT):
                        nc.tensor.matmul(
                            po, outT_sb[dt][:, ct * P:(ct + 1) * P], yT_sb[dt],
                            start=(dt == 0), stop=(dt == DT - 1))
                    t1 = outp.tile([P, QW], F32, name="t1", tag="t1")
                    nc.vector.tensor_mul(t1, po, recip)
                    t2 = outp.tile([P, QW], F32, name="t2", tag="t2")
                    nc.scalar.activation(out=t2, in_=t1, func=Ident,
                                         bias=b2_sb[ct], scale=1.0)
                    t3 = outp.tile([P, QW], F32, name="t3", tag="t3")
                    nc.vector.tensor_add(t3, t2, x_sb[ct][:, qs].bitcast(F32))
                    nc.sync.dma_start(
                        out=out_d.ap()[ct * P:(ct + 1) * P, qs], in_=t3)

    nc.compile()
    return nc


def kernel(x, g_w, g_b, theta_w, theta_b, phi_w, phi_b, out_w, out_b):
    x = np.ascontiguousarray(np.asarray(x, dtype=np.float32))
    g_w = np.asarray(g_w, dtype=np.float32)
    g_b = np.asarray(g_b, dtype=np.float32)
    theta_w = np.asarray(theta_w, dtype=np.float32)
    theta_b = np.asarray(theta_b, dtype=np.float32)
    phi_w = np.asarray(phi_w, dtype=np.float32)
    phi_b = np.asarray(phi_b, dtype=np.float32)
    out_w = np.asarray(out_w, dtype=np.float32)
    out_b = np.asarray(out_b, dtype=np.float32)

    if "nc" not in _CACHE:
        _CACHE["nc"] = _build()
    nc = _CACHE["nc"]

    thetaT = np.ascontiguousarray(theta_w.T)           # [C, CI]
    phiT = np.ascontiguousarray(phi_w.T)               # [C, CI]
    gT = np.ascontiguousarray(g_w.T)                   # [C, CI]
    outT = np.ascontiguousarray(out_w.T)               # [CI, C]
    b2 = (out_w @ g_b + out_b).reshape(C, 1).astype(np.float32)
    ones = np.ones((P, P), dtype=np.float32)

    shared = {
        "thetaT": thetaT, "phiT": phiT, "gT": gT, "outT": outT,
        "theta_b": theta_b.reshape(CI, 1).astype(np.float32),
        "phi_b": phi_b.reshape(CI, 1).astype(np.float32),
        "b2": b2, "ones": ones,
    }
    in_maps = [dict(shared, x=np.ascontiguousarray(x[b])) for b in range(B)]
    res = run_bass_kernel_spmd(nc, in_maps, core_ids=list(range(B)))
    return np.stack([res.results[b]["out"] for b in range(B)], axis=0)


# revision 4
# speedup vs baseline: 1.3826x; 1.3826x over previous
"""NonLocalBlock1D (B=8, C=512, CI=256, L=2048) on 8 trn2 NeuronCores.

Data-parallel over batch: core b computes batch element b entirely on-chip.

Per-core math (x: [C, L]):
    theta = theta_w @ x + theta_b        [CI, L]
    phi   = phi_w @ x + phi_b            [CI, L]
    gT    = x^T @ g_w^T                  [L, CI]   (g bias folded into b2)
    fT[k, q]  = sum_d phi[d, k] theta[d, q]        (logits, transposed)
    e     = exp(fT)                       (no max subtraction; logits are O(10))
    s[q]  = sum_k e[k, q]                 (via all-ones stationary matmul)
    yT[d, q] = sum_k gT[k, d] e[k, q]
    out[c, l] = (out_w^T[d, c] . yT[d, l]) / s[l] + b2[c] + x[c, l]
where b2 = out_w @ g_b + out_b  (host-precomputed).

All matmuls run in float32r (full PE rate, ~1e-4 rel err). Weights are
pre-transposed on the host so no on-device transposes are needed.
"""

import numpy as np

import concourse.bass as bass
import concourse.tile as tile
from concourse import bacc, mybir
from concourse.bass_utils import run_bass_kernel_spmd

B, C, CI, L = 8, 512, 256, 2048
P = 128
CT = C // P      # 4 c-tiles
DT = CI // P     # 2 d-tiles
KT = L // P      # 16 k-tiles
QW = 512         # q-chunk width
QC = L // QW     # 4 q-chunks

F32 = mybir.dt.float32
F32R = mybir.dt.float32r
Exp = mybir.ActivationFunctionType.Exp
Copy = mybir.ActivationFunctionType.Copy
Ident = mybir.ActivationFunctionType.Identity

_CACHE = {}


def _build():
    nc = bacc.Bacc("TRN2", target_bir_lowering=False, debug=False)

    x_d = nc.dram_tensor("x", [C, L], F32R, kind="ExternalInput")
    thetaT_d = nc.dram_tensor("thetaT", [C, CI], F32R, kind="ExternalInput")
    phiT_d = nc.dram_tensor("phiT", [C, CI], F32R, kind="ExternalInput")
    gT_d = nc.dram_tensor("gT", [C, CI], F32R, kind="ExternalInput")
    outT_d = nc.dram_tensor("outT", [CI, C], F32R, kind="ExternalInput")
    theta_b_d = nc.dram_tensor("theta_b", [CI, 1], F32, kind="ExternalInput")
    phi_b_d = nc.dram_tensor("phi_b", [CI, 1], F32, kind="ExternalInput")
    b2_d = nc.dram_tensor("b2", [C, 1], F32, kind="ExternalInput")
    ones_d = nc.dram_tensor("ones", [P, P], F32R, kind="ExternalInput")
    out_d = nc.dram_tensor("out", [C, L], F32, kind="ExternalOutput")

    with tile.TileContext(nc) as tc:
        with tc.tile_pool(name="big", bufs=1) as big, \
             tc.tile_pool(name="wpool", bufs=1) as wpool, \
             tc.tile_pool(name="expp", bufs=6) as expp, \
             tc.tile_pool(name="ytp", bufs=2) as ytp, \
             tc.tile_pool(name="outp", bufs=4) as outp, \
             tc.tile_pool(name="smallp", bufs=2) as smallp, \
             tc.tile_pool(name="ps_mm", bufs=2, space="PSUM") as ps_mm, \
             tc.tile_pool(name="ps_ft", bufs=3, space="PSUM") as ps_ft, \
             tc.tile_pool(name="ps_sy", bufs=1, space="PSUM") as ps_sy:

            # ---- load x and weights into SBUF ----
            x_sb = []
            for ct in range(CT):
                xt = big.tile([P, L], F32R, name=f"x_sb{ct}", tag=f"x{ct}")
                nc.sync.dma_start(out=xt, in_=x_d.ap()[ct * P:(ct + 1) * P, :])
                x_sb.append(xt)

            thetaT_sb, phiT_sb, gTw_sb = [], [], []
            for ct in range(CT):
                tw = wpool.tile([P, CI], F32R, name=f"thw{ct}", tag=f"thw{ct}")
                nc.sync.dma_start(out=tw, in_=thetaT_d.ap()[ct * P:(ct + 1) * P, :])
                thetaT_sb.append(tw)
                pw = wpool.tile([P, CI], F32R, name=f"phw{ct}", tag=f"phw{ct}")
                nc.sync.dma_start(out=pw, in_=phiT_d.ap()[ct * P:(ct + 1) * P, :])
                phiT_sb.append(pw)
                gw = wpool.tile([P, CI], F32R, name=f"gw{ct}", tag=f"gw{ct}")
                nc.sync.dma_start(out=gw, in_=gT_d.ap()[ct * P:(ct + 1) * P, :])
                gTw_sb.append(gw)

            outT_sb = []
            for dt in range(DT):
                ow = wpool.tile([P, C], F32R, name=f"ow{dt}", tag=f"ow{dt}")
                nc.sync.dma_start(out=ow, in_=outT_d.ap()[dt * P:(dt + 1) * P, :])
                outT_sb.append(ow)

            theta_b_sb, phi_b_sb = [], []
            for dt in range(DT):
                tb = wpool.tile([P, 1], F32, name=f"tb{dt}", tag=f"tb{dt}")
                nc.sync.dma_start(out=tb, in_=theta_b_d.ap()[dt * P:(dt + 1) * P, :])
                theta_b_sb.append(tb)
                pb = wpool.tile([P, 1], F32, name=f"pb{dt}", tag=f"pb{dt}")
                nc.sync.dma_start(out=pb, in_=phi_b_d.ap()[dt * P:(dt + 1) * P, :])
                phi_b_sb.append(pb)
            b2_sb = []
            for ct in range(CT):
                bt = wpool.tile([P, 1], F32, name=f"b2{ct}", tag=f"b2{ct}")
                nc.sync.dma_start(out=bt, in_=b2_d.ap()[ct * P:(ct + 1) * P, :])
                b2_sb.append(bt)
            ones_sb = wpool.tile([P, P], F32R, name="ones_sb", tag="ones")
            nc.sync.dma_start(out=ones_sb, in_=ones_d.ap())

            # ---- xb2 = x + b2 (for the final residual+bias add) ----
            xb2_sb = []
            for ct in range(CT):
                xb = big.tile([P, L], F32, name=f"xb2_sb{ct}", tag=f"xb2{ct}")
                nc.vector.tensor_scalar_add(xb, x_sb[ct].bitcast(F32), b2_sb[ct])
                xb2_sb.append(xb)

            # ---- projections ----
            # theta/phi: [CI, L], psum tile per (dt, chunk), contract over ct
            theta_sb = [big.tile([P, L], F32R, name=f"th_sb{dt}", tag=f"th{dt}")
                        for dt in range(DT)]
            phi_sb = [big.tile([P, L], F32R, name=f"ph_sb{dt}", tag=f"ph{dt}")
                      for dt in range(DT)]
            for dt in range(DT):
                for qc in range(QC):
                    qs = slice(qc * QW, (qc + 1) * QW)
                    pth = ps_mm.tile([P, QW], F32, name="pth", tag="mm512")
                    for ct in range(CT):
                        nc.tensor.matmul(
                            pth, thetaT_sb[ct][:, dt * P:(dt + 1) * P],
                            x_sb[ct][:, qs], start=(ct == 0), stop=(ct == CT - 1))
                    nc.vector.tensor_scalar_add(theta_sb[dt][:, qs], pth,
                                                theta_b_sb[dt])
                    pph = ps_mm.tile([P, QW], F32, name="pph", tag="mm512")
                    for ct in range(CT):
                        nc.tensor.matmul(
                            pph, phiT_sb[ct][:, dt * P:(dt + 1) * P],
                            x_sb[ct][:, qs], start=(ct == 0), stop=(ct == CT - 1))
                    nc.vector.tensor_scalar_add(phi_sb[dt][:, qs], pph,
                                                phi_b_sb[dt])

            # g_xT: [L, CI] per l-tile, x as stationary
            gT_sb = [big.tile([P, CI], F32R, name=f"gt_sb{lt}", tag=f"gt{lt}")
                     for lt in range(KT)]
            for lt in range(KT):
                pg = ps_mm.tile([P, CI], F32, name="pg", tag="mm512")
                for ct in range(CT):
                    nc.tensor.matmul(
                        pg, x_sb[ct][:, lt * P:(lt + 1) * P], gTw_sb[ct],
                        start=(ct == 0), stop=(ct == CT - 1))
                nc.vector.tensor_copy(gT_sb[lt], pg)

            # ---- attention, per q-chunk ----
            for qc in range(QC):
                qs = slice(qc * QW, (qc + 1) * QW)
                s_ps = ps_sy.tile([P, QW], F32, name="s_ps", tag="s")
                y_ps = [ps_sy.tile([P, QW], F32, name=f"y_ps{dt}", tag=f"y{dt}")
                        for dt in range(DT)]
                for kt in range(KT):
                    ks = slice(kt * P, (kt + 1) * P)
                    ft = ps_ft.tile([P, QW], F32, name="ft", tag="ft")
                    for dt in range(DT):
                        nc.tensor.matmul(ft, phi_sb[dt][:, ks],
                                         theta_sb[dt][:, qs],
                                         start=(dt == 0), stop=(dt == DT - 1))
                    ef = expp.tile([P, QW], F32R, name="ef", tag="ef")
                    nc.scalar.activation(out=ef, in_=ft, func=Exp)
                    nc.tensor.matmul(s_ps, ones_sb, ef,
                                     start=(kt == 0), stop=(kt == KT - 1))
                    for dt in range(DT):
                        nc.tensor.matmul(y_ps[dt],
                                         gT_sb[kt][:, dt * P:(dt + 1) * P], ef,
                                         start=(kt == 0), stop=(kt == KT - 1))

                recip = smallp.tile([P, QW], F32, name="recip", tag="recip")
                nc.vector.reciprocal(recip, s_ps)
                yT_sb = [ytp.tile([P, QW], F32R, name=f"yt{dt}", tag=f"yt{dt}")
                         for dt in range(DT)]
                for dt in range(DT):
                    nc.scalar.activation(out=yT_sb[dt], in_=y_ps[dt], func=Copy)

                # out projection for this q-chunk
                for ct in range(CT):
                    po = ps_mm.tile([P, QW], F32, name="po", tag="mm512")
                    for dt in range(DT):
                        nc.tensor.matmul(
                            po, outT_sb[dt][:, ct * P:(ct + 1) * P], yT_sb[dt],
                            start=(dt == 0), stop=(dt == DT - 1))
                    t1 = outp.tile([P, QW], F32, name="t1", tag="t1")
                    nc.vector.tensor_mul(t1, po, recip)
                    t3 = outp.tile([P, QW], F32, name="t3", tag="t3")
                    nc.vector.tensor_add(t3, t1, xb2_sb[ct][:, qs])
                    nc.sync.dma_start(
                        out=out_d.ap()[ct * P:(ct + 1) * P, qs], in_=t3)

    nc.compile()
    return nc


def kernel(x, g_w, g_b, theta_w, theta_b, phi_w, phi_b, out_w, out_b):
    x = np.ascontiguousarray(np.asarray(x, dtype=np.float32))
    g_w = np.asarray(g_w, dtype=np.float32)
    g_b = np.asarray(g_b, dtype=np.float32)
    theta_w = np.asarray(theta_w, dtype=np.float32)
    theta_b = np.asarray(theta_b, dtype=np.float32)
    phi_w = np.asarray(phi_w, dtype=np.float32)
    phi_b = np.asarray(phi_b, dtype=np.float32)
    out_w = np.asarray(out_w, dtype=np.float32)
    out_b = np.asarray(out_b, dtype=np.float32)

    if "nc" not in _CACHE:
        _CACHE["nc"] = _build()
    nc = _CACHE["nc"]

    thetaT = np.ascontiguousarray(theta_w.T)           # [C, CI]
    phiT = np.ascontiguousarray(phi_w.T)               # [C, CI]
    gT = np.ascontiguousarray(g_w.T)                   # [C, CI]
    outT = np.ascontiguousarray(out_w.T)               # [CI, C]
    b2 = (out_w @ g_b + out_b).reshape(C, 1).astype(np.float32)
    ones = np.ones((P, P), dtype=np.float32)

    shared = {
        "thetaT": thetaT, "phiT": phiT, "gT": gT, "outT": outT,
        "theta_b": theta_b.reshape(CI, 1).astype(np.float32),
        "phi_b": phi_b.reshape(CI, 1).astype(np.float32),
        "b2": b2, "ones": ones,
    }
    in_maps = [dict(shared, x=np.ascontiguousarray(x[b])) for b in range(B)]
    res = run_bass_kernel_spmd(nc, in_maps, core_ids=list(range(B)))
    return np.stack([res.results[b]["out"] for b in range(B)], axis=0)
